# revision 1
# baseline (speedup 1.0000x reference)
"""Trainium2 Bass kernel for AdaptiveMessagePassingLayer.

Math: out = X @ w_eff, where w_eff = sum_r scales[r] * relation_weights[r].
X: [524288, 128] f32, relation_weights: [16, 128, 128], relation_scales: [16, 1].

Sharding: data-parallel over the node dim N across 8 cores (65536 rows each).
Each shard is passed to its core transposed ([128, 65536], feature-major) so the
device streams K-major tiles straight into the TensorE with zero on-chip
transposes: out_shard.T = w_eff.T @ X_shard.T via matmul(lhsT=w_eff, rhs=xT).
The host transposes each core's [128, 65536] result back during unshard.
"""

import sys

if "/opt/trn_rl_repo" not in sys.path:
    sys.path.insert(0, "/opt/trn_rl_repo")

import numpy as np

import concourse.tile as tile
from concourse import bacc, mybir
from concourse.bass_utils import run_bass_kernel_spmd

N_CORES = 8
N_NODES = 524288
D = 128
R = 16
M = N_NODES // N_CORES  # rows per core

BLK = 4096  # X rows (xT columns) per DMA block
MMT = 512   # moving-operand tile per fp32 matmul (hardware max)

_compiled = None


def build():
    f32 = mybir.dt.float32
    nc = bacc.Bacc("TRN2", target_bir_lowering=False, debug=False,
                   num_devices=N_CORES)
    xt = nc.dram_tensor("xt", [D, M], f32, kind="ExternalInput").ap()
    rw = nc.dram_tensor("rw", [R, D, D], f32, kind="ExternalInput").ap()
    rs = nc.dram_tensor("rs", [R, 1], f32, kind="ExternalInput").ap()
    out_t = nc.dram_tensor("out_t", [D, M], f32, kind="ExternalOutput").ap()

    mult = mybir.AluOpType.mult
    add = mybir.AluOpType.add

    with tile.TileContext(nc) as tc:
        with (
            tc.tile_pool(name="const", bufs=1) as const_pool,
            tc.tile_pool(name="inp", bufs=3) as inp,
            tc.tile_pool(name="outp", bufs=3) as outp,
            tc.tile_pool(name="ps", bufs=7, space="PSUM") as ps,
            tc.tile_pool(name="pssc", bufs=1, space="PSUM") as pssc,
        ):
            # ---- w_eff = sum_r rs[r] * rw[r] ------------------------------
            # rw as [i, r, o]: partition i holds W[r, i, :] for every r.
            wtile = const_pool.tile([D, R, D], f32)
            nc.sync.dma_start(out=wtile[:], in_=rw.rearrange("r i o -> i r o"))
            sc_row = const_pool.tile([1, R], f32)
            nc.sync.dma_start(out=sc_row[:], in_=rs.rearrange("r one -> one r"))
            # Broadcast scales across partitions with a K=1 matmul of ones.
            ones = const_pool.tile([1, D], f32)
            nc.vector.memset(ones[:], 1.0)
            sc_ps = pssc.tile([D, R], f32)
            nc.tensor.matmul(out=sc_ps[:], lhsT=ones[:], rhs=sc_row[:],
                             start=True, stop=True)
            sc_b = const_pool.tile([D, R], f32)
            nc.vector.tensor_copy(out=sc_b[:], in_=sc_ps[:])

            weff = const_pool.tile([D, D], f32)
            nc.vector.tensor_scalar_mul(out=weff[:], in0=wtile[:, 0, :],
                                        scalar1=sc_b[:, 0:1])
            for r in range(1, R):
                nc.vector.scalar_tensor_tensor(
                    out=weff[:], in0=wtile[:, r, :], scalar=sc_b[:, r:r + 1],
                    in1=weff[:], op0=mult, op1=add)

            # ---- main stream: out_t[:, c] = w_eff.T @ xt[:, c] ------------
            for b in range(M // BLK):
                xin = inp.tile([D, BLK], f32)
                nc.sync.dma_start(out=xin[:], in_=xt[:, b * BLK:(b + 1) * BLK])
                xout = outp.tile([D, BLK], f32)
                for k in range(BLK // MMT):
                    pt = ps.tile([D, MMT], f32)
                    nc.tensor.matmul(out=pt[:], lhsT=weff[:],
                                     rhs=xin[:, k * MMT:(k + 1) * MMT],
                                     start=True, stop=True)
                    nc.vector.tensor_copy(out=xout[:, k * MMT:(k + 1) * MMT],
                                          in_=pt[:])
                nc.scalar.dma_start(out=out_t[:, b * BLK:(b + 1) * BLK],
                                    in_=xout[:])

    nc.compile()
    return nc


def kernel(inputs: np.ndarray, relation_weights: np.ndarray,
           relation_scales: np.ndarray) -> np.ndarray:
    global _compiled
    if _compiled is None:
        _compiled = build()
    nc = _compiled

    inputs = np.ascontiguousarray(inputs, dtype=np.float32)
    rw = np.ascontiguousarray(relation_weights, dtype=np.float32)
    rs = np.ascontiguousarray(relation_scales, dtype=np.float32)

    in_maps = []
    for i in range(N_CORES):
        shard_t = np.ascontiguousarray(inputs[i * M:(i + 1) * M].T)
        in_maps.append({"xt": shard_t, "rw": rw, "rs": rs})

    res = run_bass_kernel_spmd(nc, in_maps, core_ids=list(range(N_CORES)))

    out = np.empty((N_NODES, D), dtype=np.float32)
    for i in range(N_CORES):
        out[i * M:(i + 1) * M] = res.results[i]["out_t"].T
    return out
